# revision 1
# baseline (speedup 1.0000x reference)
"""Trainium2 Bass kernel for nn_LinkerEncoder (3-layer GCN + mean-pool + MLP
+ LayerNorm), SPMD over 8 NeuronCores.

Sharding: destinations (nodes) are partitioned across cores; each core
gather-accumulates messages for its own nodes from a replicated, degree-
normalized feature table; tables are rebuilt per layer (redundant matmul or
AllGather of the sharded layer output); per-graph pooled partial sums are
AllReduced; the tiny MLP head runs redundantly on every core.

Self-contained: generated from prep.py / hostio.py / gcn_kernel.py /
runner.py / kernel_wrapper.py by make_kernel.py.
"""
import numpy as np

# ==== from prep.py ====

N = 50000
NC = 8
SHARD = N // NC          # 6250
NBLK = (SHARD + 127) // 128   # 49
A_CORES = 5
A_SPLIT = A_CORES * SHARD     # 31250
B_PAD_IDX = N + 1 - A_SPLIT   # 18751 -> row 50001 (zero)
G = 1000
GBLK = 8                  # graph blocks
GPER = G // GBLK          # 125 graphs per block


def wrap16(stream):
    """stream [L] (L % 16 == 0) -> wrapped [16, L//16] -> replicated [128, L//16] int16."""
    L = len(stream)
    w = np.asarray(stream, np.int32).reshape(L // 16, 16).T.astype(np.int16)
    return np.tile(w, (8, 1))


def preprocess(edge_index, batch):
    src = np.asarray(edge_index[0], np.int64)
    dst = np.asarray(edge_index[1], np.int64)
    batch = np.asarray(batch, np.int64)

    indeg = np.bincount(dst, minlength=N).astype(np.int64) + 1  # + self loop
    dinv = 1.0 / np.sqrt(indeg.astype(np.float64))

    # ---- core assignment: round-robin by in-degree rank
    rank = np.argsort(-indeg, kind="stable")      # node ids, desc degree
    core_of = np.empty(N, np.int32)
    core_of[rank] = np.arange(N, dtype=np.int32) % NC

    # ---- per-dest source lists split by source core group
    # self-loop included as a regular edge (src=dst)
    all_src = np.concatenate([src, np.arange(N, dtype=np.int64)])
    all_dst = np.concatenate([dst, np.arange(N, dtype=np.int64)])
    src_is_a = core_of[all_src] < A_CORES
    a_cnt = np.bincount(all_dst, weights=src_is_a.astype(np.float64), minlength=N).astype(np.int64)
    b_cnt = np.bincount(all_dst, minlength=N) - a_cnt

    # ---- within-core ordering by (a, b)
    pos_of = np.empty(N, np.int64)
    nodes_of_core = []
    for c in range(NC):
        nodes = np.where(core_of == c)[0]
        order = np.lexsort((b_cnt[nodes], a_cnt[nodes]))
        nodes = nodes[order]
        nodes_of_core.append(nodes)
        pos_of[nodes] = SHARD * c + np.arange(SHARD)
    node_at = np.empty(N, np.int64)
    node_at[pos_of] = np.arange(N)

    # ---- per-core per-block slot counts
    SA = np.zeros((NC, NBLK), np.int64)
    SB = np.zeros((NC, NBLK), np.int64)
    for c in range(NC):
        a_sorted = a_cnt[nodes_of_core[c]]
        b_sorted = b_cnt[nodes_of_core[c]]
        for k in range(NBLK):
            lo, hi = 128 * k, min(128 * (k + 1), SHARD)
            SA[c, k] = a_sorted[lo:hi].max()
            SB[c, k] = b_sorted[lo:hi].max()
    # uniform structure across cores
    SAu = SA.max(axis=0)
    SBu = SB.max(axis=0)

    # ---- build per-core idx streams (slot-major within block)
    # edge sources sorted per dest for deterministic fill
    order = np.argsort(all_dst, kind="stable")
    s_sorted = all_src[order]
    d_sorted = all_dst[order]
    starts = np.searchsorted(d_sorted, np.arange(N))
    ends = np.searchsorted(d_sorted, np.arange(N) + 1)

    lenA = int(SAu.sum()) * 128
    lenB = int(SBu.sum()) * 128
    streamsA = np.zeros((NC, lenA), np.int32)           # pad idx 0
    streamsB = np.full((NC, lenB), B_PAD_IDX, np.int32)  # pad idx -> zero row
    for c in range(NC):
        offA = offB = 0
        for k in range(NBLK):
            lo, hi = 128 * k, min(128 * (k + 1), SHARD)
            gridA = np.zeros((int(SAu[k]), 128), np.int32)
            gridB = np.full((int(SBu[k]), 128), B_PAD_IDX, np.int32)
            for lane in range(hi - lo):
                d = nodes_of_core[c][lo + lane]
                srcs = s_sorted[starts[d]:ends[d]]
                ps = pos_of[srcs]
                pa = ps[ps < A_SPLIT] + 1
                pb = ps[ps >= A_SPLIT] + 1 - A_SPLIT
                gridA[: len(pa), lane] = pa
                gridB[: len(pb), lane] = pb
            streamsA[c, offA:offA + gridA.size] = gridA.ravel()
            streamsB[c, offB:offB + gridB.size] = gridB.ravel()
            offA += gridA.size
            offB += gridB.size

    # ---- pooling tables: graph -> member locals per core
    SP = np.zeros((NC, GBLK), np.int64)
    members = {}
    for c in range(NC):
        g_of_local = batch[nodes_of_core[c]]          # [SHARD] graph id per local
        for gb in range(GBLK):
            cnts = np.bincount(
                g_of_local[(g_of_local >= GPER * gb) & (g_of_local < GPER * (gb + 1))] - GPER * gb,
                minlength=GPER)
            SP[c, gb] = max(cnts.max(), 1)
        members[c] = g_of_local
    SPu = SP.max(axis=0)

    lenP = int(SPu.sum()) * 128
    streamsP = np.zeros((NC, lenP), np.int32)          # pad 0 -> zero row
    for c in range(NC):
        g_of_local = members[c]
        off = 0
        for gb in range(GBLK):
            grid = np.zeros((int(SPu[gb]), 128), np.int32)
            for gl in range(GPER):
                locs = np.where(g_of_local == GPER * gb + gl)[0] + 1  # 1-based
                grid[: len(locs), gl] = locs
            streamsP[c, off:off + grid.size] = grid.ravel()
            off += grid.size

    cnts = np.bincount(batch, minlength=G).astype(np.float64)
    inv_cnt = (1.0 / np.maximum(cnts, 1.0)).astype(np.float32)

    return dict(
        core_of=core_of, pos_of=pos_of, node_at=node_at,
        dinv=dinv.astype(np.float32),
        SAu=SAu, SBu=SBu, SPu=SPu,
        streamsA=streamsA, streamsB=streamsB, streamsP=streamsP,
        inv_cnt=inv_cnt,
    )


if False:
    import jax
    import reference

    inputs = reference.setup_inputs()
    P = preprocess(np.asarray(inputs["edge_index"]), np.asarray(inputs["batch"]))
    E_tot = 600000 + N
    slots = (int(P["SAu"].sum()) + int(P["SBu"].sum())) * 128 * NC
    print("SAu:", P["SAu"][:10], "... sum", P["SAu"].sum())
    print("SBu:", P["SBu"][:10], "... sum", P["SBu"].sum())
    print("SPu:", P["SPu"], "sum", P["SPu"].sum())
    print(f"total slots {slots} vs edges {E_tot}  inflation {slots/E_tot:.3f}")
    print(f"per-core stream lens: A={P['streamsA'].shape[1]} B={P['streamsB'].shape[1]} P={P['streamsP'].shape[1]}")
    la = P["streamsA"].shape[1]
    print(f"gathers/layer/core at 1024-cap: {np.ceil(la/1024) + np.ceil(P['streamsB'].shape[1]/1024)}")

# ==== from hostio.py ====

NCHUNK = 391


def make_in_maps(inputs, P):
    node_at = P["node_at"]           # position -> original node id
    dinv = P["dinv"]                 # original node order
    x = np.asarray(inputs["x"], np.float32)
    xp = x[node_at]                  # position order
    xT = np.ascontiguousarray(xp.T)  # [37, 50000]

    dinv_pos = dinv[node_at].astype(np.float32)
    dvc = np.zeros((128, NCHUNK), np.float32)
    fl = dinv_pos
    for c in range(NCHUNK):
        rows = min(128, N - 128 * c)
        dvc[:rows, c] = fl[128 * c:128 * c + rows]

    invc = np.zeros((128, GBLK), np.float32)
    for gb in range(GBLK):
        invc[:GPER, gb] = P["inv_cnt"][GPER * gb:GPER * (gb + 1)]

    def rep(v, width):
        return np.tile(np.asarray(v, np.float32)[None, :], (128, 1))

    common = dict(
        xT=xT,
        dinv_chunks=dvc,
        W1=np.asarray(inputs["W1"], np.float32),
        W2=np.asarray(inputs["W2"], np.float32),
        W3=np.asarray(inputs["W3"], np.float32),
        Wf1=np.asarray(inputs["Wf1"], np.float32),
        Wf2=np.asarray(inputs["Wf2"], np.float32),
        b1r=rep(inputs["b1"], 64), b2r=rep(inputs["b2"], 128),
        b3r=rep(inputs["b3"], 64), bf1r=rep(inputs["bf1"], 128),
        bf2r=rep(inputs["bf2"], 64), gammar=rep(inputs["gamma"], 64),
        betar=rep(inputs["beta"], 64),
        invcnt=invc,
    )

    in_maps = []
    for c in range(NC):
        dvo = np.zeros((128, NBLK), np.float32)
        own = dinv_pos[SHARD * c:SHARD * (c + 1)]
        for k in range(NBLK):
            rows = min(128, SHARD - 128 * k)
            dvo[:rows, k] = own[128 * k:128 * k + rows]
        m = dict(common)
        m["dinv_own"] = dvo
        m["idxA"] = wrap16(P["streamsA"][c])
        m["idxB"] = wrap16(P["streamsB"][c])
        m["idxP"] = wrap16(P["streamsP"][c])
        in_maps.append(m)
    return in_maps


def numpy_reference_chain(inputs, P):
    """Reference intermediates in position order, for phase-by-phase checks."""
    node_at = P["node_at"]
    pos_of = P["pos_of"]
    x = np.asarray(inputs["x"], np.float64)
    src = np.asarray(inputs["edge_index"][0], np.int64)
    dst = np.asarray(inputs["edge_index"][1], np.int64)
    batch = np.asarray(inputs["batch"], np.int64)
    dinv = P["dinv"].astype(np.float64)

    all_src = np.concatenate([src, np.arange(N)])
    all_dst = np.concatenate([dst, np.arange(N)])

    def gcn(h, W, b):
        hw = (h @ np.asarray(W, np.float64)) * dinv[:, None]
        agg = np.zeros((N, hw.shape[1]))
        np.add.at(agg, all_dst, hw[all_src])
        return np.maximum(agg * dinv[:, None] + np.asarray(b, np.float64), 0.0)

    h1 = gcn(x, inputs["W1"], inputs["b1"])
    h2 = gcn(h1, inputs["W2"], inputs["b2"])
    h3 = gcn(h2, inputs["W3"], inputs["b3"])

    sums = np.zeros((1000, 64))
    np.add.at(sums, batch, h3)
    g = sums * P["inv_cnt"][:, None].astype(np.float64)
    g = np.maximum(g @ np.asarray(inputs["Wf1"], np.float64) + np.asarray(inputs["bf1"], np.float64), 0)
    g = np.maximum(g @ np.asarray(inputs["Wf2"], np.float64) + np.asarray(inputs["bf2"], np.float64), 0)
    mu = g.mean(-1, keepdims=True)
    var = ((g - mu) ** 2).mean(-1, keepdims=True)
    out = (g - mu) / np.sqrt(var + 1e-5) * np.asarray(inputs["gamma"], np.float64) + np.asarray(inputs["beta"], np.float64)

    hw1 = ((x @ np.asarray(inputs["W1"], np.float64)) * dinv[:, None])[node_at]
    hw2 = ((h1 @ np.asarray(inputs["W2"], np.float64)) * dinv[:, None])[node_at]
    hw3 = ((h2 @ np.asarray(inputs["W3"], np.float64)) * dinv[:, None])[node_at]
    return dict(hW1p=hw1, hW2p=hw2, hW3p=hw3,
                h1p=h1[node_at], h2p=h2[node_at], h3p=h3[node_at],
                sums=sums, out=out)

# ==== from gcn_kernel.py ====
from contextlib import ExitStack

import concourse.bass as bass
import concourse.bacc as bacc
import concourse.mybir as mybir
from concourse.tile import TileContext
from concourse.masks import make_identity

F32 = mybir.dt.float32
I16 = mybir.dt.int16
AX = mybir.AxisListType
AF = mybir.ActivationFunctionType
OP = mybir.AluOpType

N = 50000
NC = 8
SHARD = 6250
NBLK = 49
A_SPLIT = 31250
G = 1000
GBLK = 8
GPER = 125
NCHUNK = 391          # ceil(50000/128), last chunk 80 rows
D_IN, D1, D2, D3 = 37, 64, 128, 64
W_MAX = 8             # max gather columns per dma_gather (1024 idx)


def _gather_grid(nc, pool, idx_sb, col0, scols, table_ap, elem, tag):
    """Gather scols columns (128 idx each) from table into a fresh tile
    [128, scols*elem]; returns the tile."""
    gt = pool.tile([128, scols * elem], F32, tag=tag)
    done = 0
    while done < scols:
        w = min(W_MAX, scols - done)
        nc.gpsimd.dma_gather(
            out_ap=gt[:, (done) * elem:(done + w) * elem].rearrange(
                "p (c f) -> p c f", c=w),
            in_ap=table_ap,
            idxs_ap=idx_sb[:, 8 * (col0 + done): 8 * (col0 + done + w)],
            num_idxs=128 * w,
            num_idxs_reg=128 * w,
            elem_size=elem,
        )
        done += w
    return gt


def _reduce_cols(nc, gt, scols, elem):
    """In-place pairwise fold of scols columns of width elem down to col 0."""
    s = scols
    while s > 1:
        h = s // 2
        nc.vector.tensor_tensor(
            out=gt[:, 0:h * elem],
            in0=gt[:, 0:h * elem],
            in1=gt[:, (s - h) * elem:s * elem],
            op=OP.add,
        )
        s -= h


def build(S, debug=False):
    """S: dict with SAu[49], SBu[49], SPu[8] (python ints)."""
    SAu, SBu, SPu = [list(map(int, S[k])) for k in ("SAu", "SBu", "SPu")]
    lenA, lenB, lenP = 128 * sum(SAu), 128 * sum(SBu), 128 * sum(SPu)

    nc = bacc.Bacc()

    # ---------------- IO ----------------
    xT = nc.dram_tensor("xT", [D_IN, N], F32, kind="ExternalInput")
    dinv_chunks = nc.dram_tensor("dinv_chunks", [128, NCHUNK], F32, kind="ExternalInput")
    dinv_own = nc.dram_tensor("dinv_own", [128, NBLK], F32, kind="ExternalInput")
    idxA = nc.dram_tensor("idxA", [128, lenA // 16], I16, kind="ExternalInput")
    idxB = nc.dram_tensor("idxB", [128, lenB // 16], I16, kind="ExternalInput")
    idxP = nc.dram_tensor("idxP", [128, lenP // 16], I16, kind="ExternalInput")
    W1 = nc.dram_tensor("W1", [D_IN, D1], F32, kind="ExternalInput")
    W2 = nc.dram_tensor("W2", [D1, D2], F32, kind="ExternalInput")
    W3 = nc.dram_tensor("W3", [D2, D3], F32, kind="ExternalInput")
    Wf1 = nc.dram_tensor("Wf1", [64, 128], F32, kind="ExternalInput")
    Wf2 = nc.dram_tensor("Wf2", [128, 64], F32, kind="ExternalInput")
    b1r = nc.dram_tensor("b1r", [128, D1], F32, kind="ExternalInput")
    b2r = nc.dram_tensor("b2r", [128, D2], F32, kind="ExternalInput")
    b3r = nc.dram_tensor("b3r", [128, D3], F32, kind="ExternalInput")
    bf1r = nc.dram_tensor("bf1r", [128, 128], F32, kind="ExternalInput")
    bf2r = nc.dram_tensor("bf2r", [128, 64], F32, kind="ExternalInput")
    gammar = nc.dram_tensor("gammar", [128, 64], F32, kind="ExternalInput")
    betar = nc.dram_tensor("betar", [128, 64], F32, kind="ExternalInput")
    invcnt = nc.dram_tensor("invcnt", [128, GBLK], F32, kind="ExternalInput")

    out = nc.dram_tensor("out", [G, 64], F32, kind="ExternalOutput")

    # internal DRAM
    hW1t = nc.dram_tensor("hW1t", [N + 2, D1], F32)
    h1own = nc.dram_tensor("h1own", [SHARD, D1], F32)
    h1t = nc.dram_tensor("h1t", [N + 2, D1], F32, addr_space="Shared")
    hW2t = nc.dram_tensor("hW2t", [N + 2, D2], F32)
    h2own = nc.dram_tensor("h2own", [SHARD, D2], F32)
    hW3own = nc.dram_tensor("hW3own", [SHARD, D3], F32)
    hW3t = nc.dram_tensor("hW3t", [N + 2, D3], F32, addr_space="Shared")
    h3ot = nc.dram_tensor("h3ot", [1 + SHARD, D3], F32)
    poolin = nc.dram_tensor("poolin", [G, 64], F32)
    pooled = nc.dram_tensor("pooled", [G, 64], F32, addr_space="Shared")

    dbg = {}
    if debug:
        for name, shape in [("d_hW1", [N + 2, D1]), ("d_h1own", [SHARD, D1]),
                            ("d_hW2", [N + 2, D2]), ("d_h2own", [SHARD, D2]),
                            ("d_hW3t", [N + 2, D3]), ("d_h3", [SHARD, D3]),
                            ("d_pooled", [G, 64])]:
            dbg[name] = nc.dram_tensor(name, shape, F32, kind="ExternalOutput")

    rg = [list(range(NC))]

    with TileContext(nc) as tc, ExitStack() as ctx:
        const = ctx.enter_context(tc.tile_pool(name="const", bufs=1))
        idxp = ctx.enter_context(tc.tile_pool(name="idxp", bufs=1))

        # ---- phase 0: constants
        ident = const.tile([128, 128], F32, tag="ident")
        make_identity(nc, ident[:])
        zt = const.tile([128, 128], F32, tag="zt")
        nc.vector.memset(zt[:], 0.0)
        epss = const.tile([128, 1], F32, tag="epss")
        nc.vector.memset(epss[:], 1e-5)

        def csb(t, p0, p1, tag):
            tl = const.tile([p0, p1], F32, tag=tag)
            nc.sync.dma_start(out=tl[:], in_=t[:])
            return tl

        W1s = csb(W1, D_IN, D1, "W1s")
        W2s = csb(W2, D1, D2, "W2s")
        W3s = csb(W3, D2, D3, "W3s")
        Wf1s = csb(Wf1, 64, 128, "Wf1s")
        Wf2s = csb(Wf2, 128, 64, "Wf2s")
        b1s = csb(b1r, 128, D1, "b1s")
        b2s = csb(b2r, 128, D2, "b2s")
        b3s = csb(b3r, 128, D3, "b3s")
        bf1s = csb(bf1r, 128, 128, "bf1s")
        bf2s = csb(bf2r, 128, 64, "bf2s")
        gams = csb(gammar, 128, 64, "gams")
        bets = csb(betar, 128, 64, "bets")
        dvc = csb(dinv_chunks, 128, NCHUNK, "dvc")
        dvo = csb(dinv_own, 128, NBLK, "dvo")
        ics = csb(invcnt, 128, GBLK, "ics")

        idxA_s = idxp.tile([128, lenA // 16], I16, tag="ia")
        nc.sync.dma_start(out=idxA_s[:], in_=idxA[:])
        idxB_s = idxp.tile([128, lenB // 16], I16, tag="ib")
        nc.sync.dma_start(out=idxB_s[:], in_=idxB[:])
        idxP_s = idxp.tile([128, lenP // 16], I16, tag="ip")
        nc.sync.dma_start(out=idxP_s[:], in_=idxP[:])

        # table guard rows
        for t, d in [(hW1t, D1), (h1t, D1), (hW2t, D2), (hW3t, D3)]:
            nc.scalar.dma_start(out=t[0:1, :], in_=zt[0:1, 0:d])
            nc.scalar.dma_start(out=t[N + 1:N + 2, :], in_=zt[0:1, 0:d])
        nc.scalar.dma_start(out=h3ot[0:1, :], in_=zt[0:1, 0:D3])

        # ---- helper: table build  dst[1+pos] = dinv[pos] * (src_row @ W)
        # via lhsT chunks  (lhsT = srcT [K, 128]) x rhs W [K, Fout]
        def table_chunk(lhsT_ap, Wsb, dst, c, rows, fout, dv_col, psum_pool, hpool):
            ps = psum_pool.tile([128, fout], F32, tag="mmps")
            nc.tensor.matmul(out=ps[:rows, :], lhsT=lhsT_ap, rhs=Wsb[:],
                             start=True, stop=True)
            hv = hpool.tile([128, fout], F32, tag="hv")
            nc.scalar.activation(hv[:rows, :], ps[:rows, :], AF.Copy, scale=dv_col)
            nc.scalar.dma_start(out=dst[1 + 128 * c:1 + 128 * c + rows, :],
                                in_=hv[:rows, :])

        # ---- phase 1: hW1 table from xT
        with tc.tile_pool(name="xtp", bufs=2) as xtp, \
             tc.tile_pool(name="ps1", bufs=2, space="PSUM") as ps1, \
             tc.tile_pool(name="hb1", bufs=3) as hb1:
            QW = 12544  # 98 chunks of 128; last quarter is 12368 = 96*128 + 80
            for q in range(4):
                w = min(QW, N - QW * q)
                xq = xtp.tile([D_IN, QW], F32, tag="xq")
                nc.sync.dma_start(out=xq[:, :w], in_=xT[:, QW * q:QW * q + w])
                for j in range((w + 127) // 128):
                    c = q * (QW // 128) + j
                    rows = min(128, N - 128 * c)
                    table_chunk(xq[:, 128 * j:128 * j + rows], W1s, hW1t, c,
                                rows, D1, dvc[:rows, c:c + 1], ps1, hb1)

        # ---- gather layer driver
        def gcn_layer(table, elem, bias_sb, h_sink, gp, gpsum=None):
            """h_sink(k, rows, h_tile) stores block k."""
            offA = offB = 0
            for k in range(NBLK):
                sa, sb_ = SAu[k], SBu[k]
                st = sa + sb_
                rows = min(128, SHARD - 128 * k)
                gt = gp.tile([128, st * elem], F32, tag="gt")
                done = 0
                while done < sa:
                    w = min(W_MAX, sa - done)
                    nc.gpsimd.dma_gather(
                        out_ap=gt[:, done * elem:(done + w) * elem].rearrange(
                            "p (c f) -> p c f", c=w),
                        in_ap=table[:],
                        idxs_ap=idxA_s[:, 8 * (offA + done):8 * (offA + done + w)],
                        num_idxs=128 * w, num_idxs_reg=128 * w, elem_size=elem)
                    done += w
                done = 0
                while done < sb_:
                    w = min(W_MAX, sb_ - done)
                    nc.gpsimd.dma_gather(
                        out_ap=gt[:, (sa + done) * elem:(sa + done + w) * elem].rearrange(
                            "p (c f) -> p c f", c=w),
                        in_ap=table[A_SPLIT:, :],
                        idxs_ap=idxB_s[:, 8 * (offB + done):8 * (offB + done + w)],
                        num_idxs=128 * w, num_idxs_reg=128 * w, elem_size=elem)
                    done += w
                offA += sa
                offB += sb_
                _reduce_cols(nc, gt, st, elem)
                # h = relu(acc * dinv + bias)
                nc.vector.tensor_tensor(
                    out=gt[:, 0:elem], in0=gt[:, 0:elem],
                    in1=dvo[:, k:k + 1].to_broadcast([128, elem]), op=OP.mult)
                nc.vector.tensor_tensor(
                    out=gt[:, 0:elem], in0=gt[:, 0:elem], in1=bias_sb[:], op=OP.add)
                nc.scalar.activation(gt[:, 0:elem], gt[:, 0:elem], AF.Relu)
                h_sink(k, rows, gt)

        # ---- phase 2: L1
        with tc.tile_pool(name="g1", bufs=3) as g1:
            def sink1(k, rows, gt):
                nc.scalar.dma_start(out=h1own[128 * k:128 * k + rows, :],
                                    in_=gt[:rows, 0:D1])
            gcn_layer(hW1t, D1, b1s, sink1, g1)

        # ---- phase 3: AllGather h1
        nc.gpsimd.collective_compute(
            "AllGather", OP.bypass, replica_groups=rg,
            ins=[h1own[:]], outs=[h1t[1:N + 1, :]])

        # ---- phase 4: hW2 table from h1t (transpose + matmul per chunk)
        with tc.tile_pool(name="h1c", bufs=3) as h1c, \
             tc.tile_pool(name="tps", bufs=2, space="PSUM") as tps, \
             tc.tile_pool(name="ps2", bufs=2, space="PSUM") as ps2, \
             tc.tile_pool(name="hb2", bufs=3) as hb2:
            for c in range(NCHUNK):
                rows = min(128, N - 128 * c)
                hc = h1c.tile([128, D1], F32, tag="hc")
                if rows < 128:
                    nc.vector.memset(hc[:], 0.0)
                nc.sync.dma_start(out=hc[:rows, :],
                                  in_=h1t[1 + 128 * c:1 + 128 * c + rows, :])
                tp = tps.tile([D1, 128], F32, tag="tp")
                nc.tensor.transpose(out=tp[:], in_=hc[:], identity=ident[:])
                hT = hb2.tile([D1, 128], F32, tag="hT")
                nc.vector.tensor_copy(out=hT[:], in_=tp[:])
                table_chunk(hT[:, :rows], W2s, hW2t, c, rows, D2,
                            dvc[:rows, c:c + 1], ps2, hb2)

        # ---- phase 5: L2
        with tc.tile_pool(name="g2", bufs=3) as g2:
            def sink2(k, rows, gt):
                nc.scalar.dma_start(out=h2own[128 * k:128 * k + rows, :],
                                    in_=gt[:rows, 0:D2])
            gcn_layer(hW2t, D2, b2s, sink2, g2)

        # ---- phase 6: hW3own from h2own
        with tc.tile_pool(name="h2c", bufs=3) as h2c, \
             tc.tile_pool(name="tps3", bufs=2, space="PSUM") as tps3, \
             tc.tile_pool(name="ps3", bufs=2, space="PSUM") as ps3, \
             tc.tile_pool(name="hb3", bufs=3) as hb3:
            for k in range(NBLK):
                rows = min(128, SHARD - 128 * k)
                hc = h2c.tile([128, D2], F32, tag="h2")
                if rows < 128:
                    nc.vector.memset(hc[:], 0.0)
                nc.sync.dma_start(out=hc[:rows, :],
                                  in_=h2own[128 * k:128 * k + rows, :])
                tp = tps3.tile([D2, 128], F32, tag="tp3")
                nc.tensor.transpose(out=tp[:], in_=hc[:], identity=ident[:])
                hT = hb3.tile([D2, 128], F32, tag="hT3")
                nc.vector.tensor_copy(out=hT[:], in_=tp[:])
                ps = ps3.tile([128, D3], F32, tag="mps3")
                nc.tensor.matmul(out=ps[:rows, :], lhsT=hT[:, :rows], rhs=W3s[:],
                                 start=True, stop=True)
                hv = hb3.tile([128, D3], F32, tag="hv3")
                nc.scalar.activation(hv[:rows, :], ps[:rows, :], AF.Copy,
                                     scale=dvo[:rows, k:k + 1])
                nc.scalar.dma_start(out=hW3own[128 * k:128 * k + rows, :],
                                    in_=hv[:rows, :])

        # ---- phase 7: AllGather hW3
        nc.gpsimd.collective_compute(
            "AllGather", OP.bypass, replica_groups=rg,
            ins=[hW3own[:]], outs=[hW3t[1:N + 1, :]])

        # ---- phase 8: L3 -> h3ot rows 1..6250
        with tc.tile_pool(name="g3", bufs=3) as g3:
            def sink3(k, rows, gt):
                nc.scalar.dma_start(out=h3ot[1 + 128 * k:1 + 128 * k + rows, :],
                                    in_=gt[:rows, 0:D3])
            gcn_layer(hW3t, D3, b3s, sink3, g3)

        # ---- phase 9: pooling partials
        with tc.tile_pool(name="gp", bufs=3) as gp:
            offP = 0
            for gb in range(GBLK):
                sp = SPu[gb]
                gt = gp.tile([128, sp * 64], F32, tag="gtp")
                done = 0
                while done < sp:
                    w = min(W_MAX, sp - done)
                    nc.gpsimd.dma_gather(
                        out_ap=gt[:, done * 64:(done + w) * 64].rearrange(
                            "p (c f) -> p c f", c=w),
                        in_ap=h3ot[:],
                        idxs_ap=idxP_s[:, 8 * (offP + done):8 * (offP + done + w)],
                        num_idxs=128 * w, num_idxs_reg=128 * w, elem_size=64)
                    done += w
                offP += sp
                _reduce_cols(nc, gt, sp, 64)
                nc.scalar.dma_start(out=poolin[GPER * gb:GPER * (gb + 1), :],
                                    in_=gt[:GPER, 0:64])

        # ---- phase 10: AllReduce pooled sums
        nc.gpsimd.collective_compute(
            "AllReduce", OP.add, replica_groups=rg,
            ins=[poolin[:]], outs=[pooled[:]])

        # ---- phase 11: MLP + LayerNorm (redundant on every core)
        with tc.tile_pool(name="mlp", bufs=2) as mlp, \
             tc.tile_pool(name="mps", bufs=2, space="PSUM") as mps:
            for gb in range(GBLK):
                gtl = mlp.tile([128, 64], F32, tag="g0")
                nc.vector.memset(gtl[:], 0.0)
                nc.sync.dma_start(out=gtl[:GPER, :],
                                  in_=pooled[GPER * gb:GPER * (gb + 1), :])
                # mean pool scale
                nc.vector.tensor_tensor(
                    out=gtl[:], in0=gtl[:],
                    in1=ics[:, gb:gb + 1].to_broadcast([128, 64]), op=OP.mult)
                # dense1: relu(g @ Wf1 + bf1)
                tp = mps.tile([64, 128], F32, tag="t1")
                nc.tensor.transpose(out=tp[:], in_=gtl[:], identity=ident[:])
                gT = mlp.tile([64, 128], F32, tag="gT")
                nc.vector.tensor_copy(out=gT[:], in_=tp[:])
                p1 = mps.tile([128, 128], F32, tag="p1")
                nc.tensor.matmul(out=p1[:], lhsT=gT[:], rhs=Wf1s[:],
                                 start=True, stop=True)
                g1t = mlp.tile([128, 128], F32, tag="g1t")
                nc.vector.tensor_tensor(out=g1t[:], in0=p1[:], in1=bf1s[:], op=OP.add)
                nc.scalar.activation(g1t[:], g1t[:], AF.Relu)
                # dense2: relu(g1 @ Wf2 + bf2)
                tp2 = mps.tile([128, 128], F32, tag="t2")
                nc.tensor.transpose(out=tp2[:], in_=g1t[:], identity=ident[:])
                g1T = mlp.tile([128, 128], F32, tag="g1T")
                nc.vector.tensor_copy(out=g1T[:], in_=tp2[:])
                p2 = mps.tile([128, 64], F32, tag="p2")
                nc.tensor.matmul(out=p2[:], lhsT=g1T[:], rhs=Wf2s[:],
                                 start=True, stop=True)
                g2t = mlp.tile([128, 64], F32, tag="g2t")
                nc.vector.tensor_tensor(out=g2t[:], in0=p2[:], in1=bf2s[:], op=OP.add)
                nc.scalar.activation(g2t[:], g2t[:], AF.Relu)
                # layernorm over 64 features
                mu = mlp.tile([128, 1], F32, tag="mu")
                nc.vector.reduce_sum(mu[:], g2t[:], axis=AX.X)
                nc.vector.tensor_scalar_mul(mu[:], in0=mu[:], scalar1=1.0 / 64)
                xm = mlp.tile([128, 64], F32, tag="xm")
                nc.vector.tensor_tensor(out=xm[:], in0=g2t[:],
                                        in1=mu[:].to_broadcast([128, 64]),
                                        op=OP.subtract)
                sq = mlp.tile([128, 64], F32, tag="sq")
                nc.vector.tensor_tensor(out=sq[:], in0=xm[:], in1=xm[:], op=OP.mult)
                var = mlp.tile([128, 1], F32, tag="var")
                nc.vector.reduce_sum(var[:], sq[:], axis=AX.X)
                rstd = mlp.tile([128, 1], F32, tag="rstd")
                nc.vector.tensor_scalar_mul(var[:], in0=var[:], scalar1=1.0 / 64)
                nc.vector.tensor_tensor(out=var[:], in0=var[:], in1=epss[:],
                                        op=OP.add)
                nc.scalar.activation(rstd[:], var[:], AF.Sqrt)
                nc.vector.reciprocal(rstd[:], rstd[:])
                nc.vector.tensor_tensor(out=xm[:], in0=xm[:],
                                        in1=rstd[:].to_broadcast([128, 64]),
                                        op=OP.mult)
                nc.vector.tensor_tensor(out=xm[:], in0=xm[:], in1=gams[:], op=OP.mult)
                nc.vector.tensor_tensor(out=xm[:], in0=xm[:], in1=bets[:], op=OP.add)
                nc.sync.dma_start(out=out[GPER * gb:GPER * (gb + 1), :],
                                  in_=xm[:GPER, :])

        # ---- debug dumps
        if debug:
            with tc.tile_pool(name="dbg", bufs=2) as dp:
                def dump(src, dst, nrows, width):
                    for c in range((nrows + 127) // 128):
                        rows = min(128, nrows - 128 * c)
                        t = dp.tile([128, width], F32, tag="dt")
                        nc.sync.dma_start(out=t[:rows, :],
                                          in_=src[128 * c:128 * c + rows, :])
                        nc.sync.dma_start(out=dst[128 * c:128 * c + rows, :],
                                          in_=t[:rows, :])
                dump(hW1t, dbg["d_hW1"], N + 2, D1)
                dump(h1own, dbg["d_h1own"], SHARD, D1)
                dump(hW2t, dbg["d_hW2"], N + 2, D2)
                dump(h2own, dbg["d_h2own"], SHARD, D2)
                dump(hW3t, dbg["d_hW3t"], N + 2, D3)
                dump(h3ot[1:, :], dbg["d_h3"], SHARD, D3)
                dump(pooled, dbg["d_pooled"], G, 64)

    nc.compile()
    nc.finalize()
    return nc

# ==== from runner.py ====
import jax
from jax.sharding import Mesh, PartitionSpec
from jax.experimental.shard_map import shard_map

from concourse import bass, bass2jax, mybir


class SpmdRunner:
    def __init__(self, nc, n_cores=8):
        bass2jax.install_neuronx_cc_hook()
        self.nc = nc
        self.n_cores = n_cores
        partition_name = nc.partition_id_tensor.name if nc.partition_id_tensor else None
        in_names, out_names, out_avals, zero_outs = [], [], [], []
        for alloc in nc.m.functions[0].allocations:
            if not isinstance(alloc, mybir.MemoryLocationSet):
                continue
            name = alloc.memorylocations[0].name
            if alloc.kind == "ExternalInput":
                if name != partition_name:
                    in_names.append(name)
            elif alloc.kind == "ExternalOutput":
                shape = tuple(alloc.tensor_shape)
                dtype = mybir.dt.np(alloc.dtype)
                out_names.append(name)
                out_avals.append(jax.core.ShapedArray(shape, dtype))
                zero_outs.append(np.zeros(shape, dtype))
        self.in_names = list(in_names)
        self.out_names = out_names
        self.out_avals = out_avals
        self.zero_outs = zero_outs
        n_params = len(in_names)
        n_outs = len(out_avals)
        all_in_names = in_names + out_names + ([partition_name] if partition_name else [])
        self.n_params = n_params

        def _body(*args):
            operands = list(args)
            if partition_name is not None:
                operands.append(bass2jax.partition_id_tensor())
            outs = bass2jax._bass_exec_p.bind(
                *operands,
                out_avals=tuple(out_avals),
                in_names=tuple(all_in_names),
                out_names=tuple(out_names),
                lowering_input_output_aliases=(),
                sim_require_finite=True,
                sim_require_nnan=True,
                nc=nc,
            )
            return tuple(outs)

        try:
            devices = jax.devices("axon")[:n_cores]
        except RuntimeError:
            devices = jax.devices()[:n_cores]
        mesh = Mesh(np.asarray(devices), ("core",))
        in_specs = (PartitionSpec("core"),) * (n_params + n_outs)
        out_specs = (PartitionSpec("core"),) * n_outs
        # no donation: lets us call repeatedly with the same device arrays
        self.fn = jax.jit(
            shard_map(_body, mesh=mesh, in_specs=in_specs, out_specs=out_specs,
                      check_rep=False),
            keep_unused=True,
        )

    def stage(self, in_maps):
        """Concat per-core inputs to global arrays (host)."""
        concat = [
            np.concatenate([np.asarray(in_maps[c][n]) for c in range(self.n_cores)], axis=0)
            for n in self.in_names
        ]
        zeros = [np.zeros((self.n_cores * z.shape[0], *z.shape[1:]), z.dtype)
                 for z in self.zero_outs]
        return concat + zeros

    def run(self, staged):
        out = self.fn(*staged)
        jax.block_until_ready(out)
        return out

    def unpack(self, out_arrs):
        return [
            {
                name: np.asarray(out_arrs[i]).reshape(
                    self.n_cores, *self.out_avals[i].shape)[c]
                for i, name in enumerate(self.out_names)
            }
            for c in range(self.n_cores)
        ]

# ==== from kernel_wrapper.py ====
# ---- public entry point -----------------------------------------------------
_CACHE = {}


def kernel(**inputs):
    """Full-input GCN encoder on 8 NeuronCores; returns [1000, 64] float32."""
    inputs = {k: np.asarray(v) for k, v in inputs.items()}
    P = preprocess(inputs["edge_index"], inputs["batch"])
    key = (tuple(P["SAu"].tolist()), tuple(P["SBu"].tolist()),
           tuple(P["SPu"].tolist()))
    if key not in _CACHE:
        S = dict(SAu=P["SAu"], SBu=P["SBu"], SPu=P["SPu"])
        nc = build(S, debug=False)
        _CACHE[key] = SpmdRunner(nc, 8)
    r = _CACHE[key]
    in_maps = make_in_maps(inputs, P)
    staged = r.stage(in_maps)
    res = r.unpack(r.run(staged))
    return np.ascontiguousarray(res[0]["out"], dtype=np.float32)

